# revision 33
# baseline (speedup 1.0000x reference)
"""2-layer dense GCN on 8 Trainium2 NeuronCores — fp8 A-stream + DoubleRow.

Reference computation (all fp32):
    H0 = relu((A_norm @ X) @ W0)
    H1 = relu((A_norm @ H0) @ W1)
A_norm: [16384, 16384], X: [16384, 128], W0/W1: [128, 128].

Sharding: 1D row partition of A_norm (2048 rows/core). Each core holds
A[rows_c].T host-quantized to fp8-e4m3 (x 2^16 scale, folded back via
the W matrices), streamed over HBM at 1 byte/element — half the DMA of
bf16, which is the dominant traffic.

Precision scheme (sim: rel err 2.3e-3 vs 2e-2 budget):
  layer 0: stationary X bf16  x  moving A e4m3   (mixed-dtype matmul)
  layer 1: stationary H e4m3  x  moving A e4m3   (DoubleRow, 2 MACs/cell)
X quantized to fp8 would cost ~1.5e-2 (zero-mean cancellation amplifies
quantization noise) so X stays bf16; H is post-relu/positive and cheap
to quantize, enabling DoubleRow's 2x PE rate for the whole second layer.

Schedule (per layer, per core): the 2048 output rows split into 4
chunks of 512 (one PSUM bank each). A^T is host-tiled chunk-major so
layer 0 finishes chunk k at ~(k+1)/4 of the layer and AllGathers it
immediately, overlapping remaining compute. Layer 1 runs piece-outer
(all chunks' piece-p matmuls per round) so gather chunk p is only
needed at round p — each gather gets a ~1/4-layer deadline slack and
never stalls the PE.

Aggregate matmul:  psum[d, i] += H_tile[q, d].T @ A^T[q, i]
Linear matmul:     psum[i, e]  = M[d, i-slice].T @ W[d, e]  (fp32r)
Relu fused into the PSUM eviction on the scalar engine (H scale 2^8
folded into W0 so the bf16->e4m3 convert is a plain relu+cast).
"""

import sys
from contextlib import ExitStack

if "/opt/trn_rl_repo" not in sys.path:
    sys.path.insert(0, "/opt/trn_rl_repo")

import numpy as np

N_NODES = 16384
D = 128
NCORES = 8
ROWS = N_NODES // NCORES   # 2048 output rows per core
NCH = 4                    # output column chunks (512 wide, 1 PSUM bank)
NPIECE = 4                 # j-piece blocks per rank (gather granularity)
NT = 4                     # j-tiles per piece
IC = 512                   # chunk width
NG = NCH * NPIECE          # A DMA groups per layer (2 MiB each)
NB = NCORES * NT           # j-tile blocks per A group
S_A = float(2 ** 16)       # fp8 pre-scale for A (max |A|*S_A ~ 8.1)
S_H = float(2 ** 8)        # fp8 pre-scale for H (max ~12)

PRECISION = "fp8dr"


def build_gcn():
    """Build the SPMD Bass program (one program, runs on all cores)."""
    import concourse.bass as bass  # noqa: F401
    import concourse.tile as tile
    from concourse import bacc, mybir

    F32 = mybir.dt.float32
    F32R = mybir.dt.float32r
    BF16 = mybir.dt.bfloat16
    E4 = mybir.dt.float8e4
    relu = mybir.ActivationFunctionType.Relu
    DR = mybir.MatmulPerfMode.DoubleRow

    nc = bacc.Bacc("TRN2", target_bir_lowering=False, num_devices=NCORES)

    # A^T shard, host pre-tiled chunk-major (see shard_inputs):
    # group g=(ch*NPIECE+p) is rows [g*128, (g+1)*128) with
    # a_in[g*128+q, (r*NT+t)*IC + i] = A^T[(r*16+p*NT+t)*128+q, ch*IC+i]
    a_in = nc.dram_tensor("a0", [NG * 128, NB * IC], E4, kind="ExternalInput")
    # x_in[r*128+p, tl*128+dd] = X[(r*16+tl)*128+p, dd]  (bf16)
    x_in = nc.dram_tensor("x0", [NCORES * 128, ROWS], BF16, kind="ExternalInput")
    w0 = nc.dram_tensor("w0", [D, D], F32R, kind="ExternalInput")
    w1 = nc.dram_tensor("w1", [D, D], F32R, kind="ExternalInput")
    h_out = nc.dram_tensor("h_out", [ROWS, D], F32, kind="ExternalOutput")

    with tile.TileContext(nc) as tc, ExitStack() as ctx:
        sb1 = ctx.enter_context(tc.tile_pool(name="sb1", bufs=1))
        statx_pool = ctx.enter_context(tc.tile_pool(name="sx", bufs=NCORES))
        stath_pool = ctx.enter_context(
            tc.tile_pool(name="sh", bufs=NCORES * NPIECE)
        )
        a_pool = ctx.enter_context(tc.tile_pool(name="a", bufs=9))
        m_pool = ctx.enter_context(tc.tile_pool(name="m", bufs=2))
        h_pool = ctx.enter_context(tc.tile_pool(name="h", bufs=12))
        agg_pool = ctx.enter_context(tc.tile_pool(name="agg", bufs=4, space="PSUM"))
        lin_pool = ctx.enter_context(tc.tile_pool(name="lin", bufs=3, space="PSUM"))
        dram = ctx.enter_context(tc.tile_pool(name="dram", bufs=1, space="DRAM"))

        w0_sb = sb1.tile([D, D], F32R)
        nc.scalar.dma_start(out=w0_sb[:], in_=w0[:])
        w1_sb = sb1.tile([D, D], F32R)
        nc.scalar.dma_start(out=w1_sb[:], in_=w1[:])

        # stationary X on the fast hwdge queues (gpsimd stays free for the
        # h writes + gather triggers). Ranks 0/1 load ahead of the first A
        # groups; ranks 2..7 are emitted just after them (see layer()) so
        # the first matmul isn't stuck behind 4 MB of X.
        statx = []
        for r in range(NCORES):
            statx.append(statx_pool.tile([128, ROWS], BF16, name=f"sx{r}", tag="sx"))

        def load_statx(r):
            eng = nc.sync if r % 2 == 0 else nc.scalar
            eng.dma_start(out=statx[r][:], in_=x_in[r * 128 : (r + 1) * 128, :])

        for _r in range(4):
            load_statx(_r)

        # hidden-state bounce + gather buffers, one per chunk (e4m3)
        h_tb = [dram.tile([128, IC], E4, name=f"htb{c}") for c in range(NCH)]
        h_ag = [
            dram.tile([NCORES * 128, IC], E4, addr_space="Shared", name=f"hag{c}")
            for c in range(NCH)
        ]
        # gathered H as stationary tiles, one per (rank, piece); 3D so
        # DoubleRow can take k-subtile pairs on dim 1
        stath = [
            [
                stath_pool.tile([128, NT, 128], E4, name=f"sh{r}_{p}", tag="sh")
                for p in range(NPIECE)
            ]
            for r in range(NCORES)
        ]

        def layer(w_sb, write_out, chunk_done, piece_outer, double_row, extra_dma):
            # schedule: L0 chunk-outer (finish chunks early -> early gathers);
            # L1 piece-outer (piece-p rounds start ~1/4-layer apart, so each
            # gather chunk has a late deadline and never stalls the PE)
            if piece_outer:
                sched = [(ch, p) for p in range(NPIECE) for ch in range(NCH)]
            else:
                sched = [(ch, p) for ch in range(NCH) for p in range(NPIECE)]
            aggs = {}
            ats = {}

            def issue_group(si):
                ch, p = sched[si]
                g = ch * NPIECE + p
                at = a_pool.tile([128, NB, IC], E4, name="at", tag="at")
                if not piece_outer and si < 2:
                    # first two groups split across both queues so the PE
                    # starts ~6us earlier instead of waiting a full 2 MiB
                    half = NB // 2 * IC
                    nc.sync.dma_start(
                        out=at[:, : NB // 2, :],
                        in_=a_in[g * 128 : (g + 1) * 128, :half],
                    )
                    nc.scalar.dma_start(
                        out=at[:, NB // 2 :, :],
                        in_=a_in[g * 128 : (g + 1) * 128, half:],
                    )
                else:
                    eng = nc.sync if si % 2 == 0 else nc.scalar
                    eng.dma_start(out=at[:], in_=a_in[g * 128 : (g + 1) * 128, :])
                ats[si] = at

            for si, (ch, p) in enumerate(sched):
                if p == 0:
                    aggs[ch] = agg_pool.tile([128, IC], F32, name=f"agg{ch}", tag="agg")
                agg = aggs[ch]
                if si not in ats:
                    issue_group(si)
                at = ats.pop(si)
                extra_dma(si, ch, p, issue_group)
                if double_row:
                    for r in range(NCORES):
                        for u in range(NT // 2):
                            nc.tensor.matmul(
                                agg[:],
                                lhsT=stath[r][p][:, 2 * u : 2 * u + 2, :],
                                rhs=at[:, r * NT + 2 * u : r * NT + 2 * u + 2, :],
                                start=(p == 0 and r == 0 and u == 0),
                                stop=(
                                    p == NPIECE - 1
                                    and r == NCORES - 1
                                    and u == NT // 2 - 1
                                ),
                                perf_mode=DR,
                            )
                else:
                    for r in range(NCORES):
                        for t in range(NT):
                            jr = p * NT + t
                            nc.tensor.matmul(
                                agg[:],
                                lhsT=statx[r][:, jr * 128 : (jr + 1) * 128],
                                rhs=at[:, r * NT + t : r * NT + t + 1, :],
                                start=(p == 0 and r == 0 and t == 0),
                                stop=(
                                    p == NPIECE - 1
                                    and r == NCORES - 1
                                    and t == NT - 1
                                ),
                            )
                if p == NPIECE - 1:
                    mt = m_pool.tile([128, IC], F32R, name="mt", tag="mt")
                    nc.vector.tensor_copy(out=mt[:], in_=agg[:])
                    for it in range(IC // 128):
                        lp = lin_pool.tile([128, D], F32, name="lp", tag="lp")
                        nc.tensor.matmul(
                            lp[:],
                            lhsT=mt[:, it * 128 : (it + 1) * 128],
                            rhs=w_sb[:],
                            start=True,
                            stop=True,
                        )
                        write_out(ch, it, lp)
                    chunk_done(ch)

        # ---- layer 0 ----
        def write_l0(ch, it, lp):
            ht = h_pool.tile([128, D], E4, name="ht0", tag="ht0")
            nc.scalar.activation(ht[:], lp[:], relu)
            # scalar hwdge queue: the gpsimd queue carries the CC doorbell
            # writes, which can block on a backed-up CC stream (slow cross-
            # core barrier) and would stall ht recycling -> the PE stream
            nc.scalar.dma_start(out=h_tb[ch][:, it * 128 : (it + 1) * 128], in_=ht[:])

        def gather(ch):
            nc.gpsimd.collective_compute(
                "AllGather",
                mybir.AluOpType.bypass,
                replica_groups=[list(range(NCORES))],
                ins=[h_tb[ch][:]],
                outs=[h_ag[ch][:]],
            )

        def extra_l0(si, ch, p, issue_group):
            # interleave the remaining X loads with the split first groups:
            # queue order sync = sx0,sx2,g0h,sx4,g1h,sx6,g2..., mirrored on
            # scalar — supplies track the PE's rank-by-rank demand in block 0
            if si == 0:
                load_statx(4)
                load_statx(5)
                issue_group(1)
                load_statx(6)
                load_statx(7)

        layer(w0_sb, write_l0, gather, piece_outer=False, double_row=False,
              extra_dma=extra_l0)

        # ---- layer 1 ----
        def write_l1(ch, it, lp):
            ht = h_pool.tile([128, D], F32, name="ht1", tag="ht1")
            nc.scalar.activation(ht[:], lp[:], relu)
            # gpsimd is idle by round 3 (stath loads done); keeps the final
            # output writes off the A-stream queues
            nc.gpsimd.dma_start(
                out=h_out[ch * IC + it * 128 : ch * IC + (it + 1) * 128, :], in_=ht[:]
            )

        # stationary-H piece loads, gpsimd only, emitted AFTER every gather
        # trigger: a piece-p load waits on gather-p completing, and putting
        # any such wait between h writes and a later gather trigger (or on
        # the A-stream queues) stalls that path on straggler cores
        for p in range(NPIECE):
            for r in range(NCORES):
                nc.gpsimd.dma_start(
                    out=stath[r][p][:], in_=h_ag[p][r * 128 : (r + 1) * 128, :]
                )

        layer(w1_sb, write_l1, lambda ch: None, piece_outer=True, double_row=True,
              extra_dma=lambda si, ch, p, issue_group: None)

    nc.finalize()
    return nc


def _tile_stat(X):
    """[16384, 128] -> [1024, 2048] stationary layout."""
    return np.ascontiguousarray(
        X.reshape(NCORES, 16, 128, D).transpose(0, 2, 1, 3).reshape(NCORES * 128, ROWS)
    )


def shard_inputs(A_norm, X, W0, W1):
    """Host-side shard prep. Returns per-core input maps."""
    import ml_dtypes

    bf16 = ml_dtypes.bfloat16
    e4 = ml_dtypes.float8_e4m3

    x_t = _tile_stat(np.asarray(X, np.float32)).astype(bf16)
    # fold the fp8 pre-scales into the weights:
    #   psum_l0 = S_A*(A@X);      w0' = W0*S_H/S_A  -> h_tb = S_H*H0 (e4m3)
    #   psum_l1 = S_A*S_H*(A@H0); w1' = W1/(S_A*S_H)
    w0 = np.ascontiguousarray(np.asarray(W0, np.float32) * np.float32(S_H / S_A))
    w1 = np.ascontiguousarray(np.asarray(W1, np.float32) / np.float32(S_A * S_H))

    in_maps = []
    for c in range(NCORES):
        a_tc = np.asarray(A_norm[c * ROWS : (c + 1) * ROWS, :], np.float32).T
        a8 = np.clip(a_tc * np.float32(S_A), 0.0, 240.0).astype(e4)
        # [16384, 2048] -> chunk-major groups (see a_in comment)
        a_pre = np.ascontiguousarray(
            a8.reshape(NCORES, NPIECE, NT, 128, NCH, IC)
            .transpose(4, 1, 3, 0, 2, 5)
            .reshape(NG * 128, NB * IC)
        )
        in_maps.append({"a0": a_pre, "x0": x_t, "w0": w0, "w1": w1})
    return in_maps


_CACHED = {}


def kernel(A_norm, X, W0, W1):
    A_norm = np.ascontiguousarray(A_norm, dtype=np.float32)
    X = np.ascontiguousarray(X, dtype=np.float32)
    W0 = np.ascontiguousarray(W0, dtype=np.float32)
    W1 = np.ascontiguousarray(W1, dtype=np.float32)

    from concourse.bass_utils import run_bass_kernel_spmd

    if PRECISION not in _CACHED:
        _CACHED[PRECISION] = build_gcn()
    nc = _CACHED[PRECISION]

    in_maps = shard_inputs(A_norm, X, W0, W1)
    res = run_bass_kernel_spmd(nc, in_maps, core_ids=list(range(NCORES)))
    return np.concatenate([res.results[c]["h_out"] for c in range(NCORES)], axis=0)


# revision 34
# speedup vs baseline: 1.0012x; 1.0012x over previous
"""2-layer dense GCN on 8 Trainium2 NeuronCores — fp8 A-stream + DoubleRow.

Reference computation (all fp32):
    H0 = relu((A_norm @ X) @ W0)
    H1 = relu((A_norm @ H0) @ W1)
A_norm: [16384, 16384], X: [16384, 128], W0/W1: [128, 128].

Sharding: 1D row partition of A_norm (2048 rows/core). Each core holds
A[rows_c].T host-quantized to fp8-e4m3 (x 2^16 scale, folded back via
the W matrices), streamed over HBM at 1 byte/element — half the DMA of
bf16, which is the dominant traffic.

Precision scheme (sim: rel err 2.3e-3 vs 2e-2 budget):
  layer 0: stationary X bf16  x  moving A e4m3   (mixed-dtype matmul)
  layer 1: stationary H e4m3  x  moving A e4m3   (DoubleRow, 2 MACs/cell)
X quantized to fp8 would cost ~1.5e-2 (zero-mean cancellation amplifies
quantization noise) so X stays bf16; H is post-relu/positive and cheap
to quantize, enabling DoubleRow's 2x PE rate for the whole second layer.

Schedule (per layer, per core): the 2048 output rows split into 4
chunks of 512 (one PSUM bank each). A^T is host-tiled chunk-major so
layer 0 finishes chunk k at ~(k+1)/4 of the layer and AllGathers it
immediately, overlapping remaining compute. Layer 1 runs piece-outer
(all chunks' piece-p matmuls per round) so gather chunk p is only
needed at round p — each gather gets a ~1/4-layer deadline slack and
never stalls the PE.

Aggregate matmul:  psum[d, i] += H_tile[q, d].T @ A^T[q, i]
Linear matmul:     psum[i, e]  = M[d, i-slice].T @ W[d, e]  (fp32r)
Relu fused into the PSUM eviction on the scalar engine (H scale 2^8
folded into W0 so the bf16->e4m3 convert is a plain relu+cast).
"""

import sys
from contextlib import ExitStack

if "/opt/trn_rl_repo" not in sys.path:
    sys.path.insert(0, "/opt/trn_rl_repo")

import numpy as np

N_NODES = 16384
D = 128
NCORES = 8
ROWS = N_NODES // NCORES   # 2048 output rows per core
NCH = 4                    # output column chunks (512 wide, 1 PSUM bank)
NPIECE = 4                 # j-piece blocks per rank (gather granularity)
NT = 4                     # j-tiles per piece
IC = 512                   # chunk width
NG = NCH * NPIECE          # A DMA groups per layer (2 MiB each)
NB = NCORES * NT           # j-tile blocks per A group
S_A = float(2 ** 16)       # fp8 pre-scale for A (max |A|*S_A ~ 8.1)
S_H = float(2 ** 8)        # fp8 pre-scale for H (max ~12)

PRECISION = "fp8dr"


def build_gcn():
    """Build the SPMD Bass program (one program, runs on all cores)."""
    import concourse.bass as bass  # noqa: F401
    import concourse.tile as tile
    from concourse import bacc, mybir

    F32 = mybir.dt.float32
    F32R = mybir.dt.float32r
    BF16 = mybir.dt.bfloat16
    E4 = mybir.dt.float8e4
    relu = mybir.ActivationFunctionType.Relu
    DR = mybir.MatmulPerfMode.DoubleRow

    nc = bacc.Bacc("TRN2", target_bir_lowering=False, num_devices=NCORES)

    # A^T shard, host pre-tiled chunk-major (see shard_inputs):
    # group g=(ch*NPIECE+p) is rows [g*128, (g+1)*128) with
    # a_in[g*128+q, (r*NT+t)*IC + i] = A^T[(r*16+p*NT+t)*128+q, ch*IC+i]
    a_in = nc.dram_tensor("a0", [NG * 128, NB * IC], E4, kind="ExternalInput")
    # x_in[r*128+p, tl*128+dd] = X[(r*16+tl)*128+p, dd]  (bf16)
    x_in = nc.dram_tensor("x0", [NCORES * 128, ROWS], BF16, kind="ExternalInput")
    w0 = nc.dram_tensor("w0", [D, D], F32R, kind="ExternalInput")
    w1 = nc.dram_tensor("w1", [D, D], F32R, kind="ExternalInput")
    h_out = nc.dram_tensor("h_out", [ROWS, D], F32, kind="ExternalOutput")

    with tile.TileContext(nc) as tc, ExitStack() as ctx:
        sb1 = ctx.enter_context(tc.tile_pool(name="sb1", bufs=1))
        statx_pool = ctx.enter_context(tc.tile_pool(name="sx", bufs=NCORES))
        stath_pool = ctx.enter_context(
            tc.tile_pool(name="sh", bufs=NCORES * NPIECE)
        )
        a_pool = ctx.enter_context(tc.tile_pool(name="a", bufs=9))
        m_pool = ctx.enter_context(tc.tile_pool(name="m", bufs=2))
        h_pool = ctx.enter_context(tc.tile_pool(name="h", bufs=12))
        agg_pool = ctx.enter_context(tc.tile_pool(name="agg", bufs=4, space="PSUM"))
        lin_pool = ctx.enter_context(tc.tile_pool(name="lin", bufs=3, space="PSUM"))
        dram = ctx.enter_context(tc.tile_pool(name="dram", bufs=1, space="DRAM"))

        w0_sb = sb1.tile([D, D], F32R)
        nc.scalar.dma_start(out=w0_sb[:], in_=w0[:])
        w1_sb = sb1.tile([D, D], F32R)
        nc.scalar.dma_start(out=w1_sb[:], in_=w1[:])

        # stationary X on the fast hwdge queues (gpsimd stays free for the
        # h writes + gather triggers). Ranks 0/1 load ahead of the first A
        # groups; ranks 2..7 are emitted just after them (see layer()) so
        # the first matmul isn't stuck behind 4 MB of X.
        statx = []
        for r in range(NCORES):
            statx.append(statx_pool.tile([128, ROWS], BF16, name=f"sx{r}", tag="sx"))

        def load_statx(r):
            eng = nc.sync if r % 2 == 0 else nc.scalar
            eng.dma_start(out=statx[r][:], in_=x_in[r * 128 : (r + 1) * 128, :])

        for _r in range(4):
            load_statx(_r)

        # hidden-state bounce + gather buffers, one per chunk (e4m3)
        h_tb = [dram.tile([128, IC], E4, name=f"htb{c}") for c in range(NCH)]
        h_ag = [
            dram.tile([NCORES * 128, IC], E4, addr_space="Shared", name=f"hag{c}")
            for c in range(NCH)
        ]
        # gathered H as stationary tiles, one per (rank, piece); 3D so
        # DoubleRow can take k-subtile pairs on dim 1
        stath = [
            [
                stath_pool.tile([128, NT, 128], E4, name=f"sh{r}_{p}", tag="sh")
                for p in range(NPIECE)
            ]
            for r in range(NCORES)
        ]

        def layer(w_sb, write_out, chunk_done, piece_outer, double_row, extra_dma):
            # schedule: L0 chunk-outer (finish chunks early -> early gathers);
            # L1 piece-outer (piece-p rounds start ~1/4-layer apart, so each
            # gather chunk has a late deadline and never stalls the PE)
            if piece_outer:
                sched = [(ch, p) for p in range(NPIECE) for ch in range(NCH)]
            else:
                sched = [(ch, p) for ch in range(NCH) for p in range(NPIECE)]
            aggs = {}
            ats = {}

            def issue_group(si):
                ch, p = sched[si]
                g = ch * NPIECE + p
                at = a_pool.tile([128, NB, IC], E4, name="at", tag="at")
                if not piece_outer and si < 2:
                    # first two groups split across both queues so the PE
                    # starts ~6us earlier instead of waiting a full 2 MiB
                    half = NB // 2 * IC
                    nc.sync.dma_start(
                        out=at[:, : NB // 2, :],
                        in_=a_in[g * 128 : (g + 1) * 128, :half],
                    )
                    nc.scalar.dma_start(
                        out=at[:, NB // 2 :, :],
                        in_=a_in[g * 128 : (g + 1) * 128, half:],
                    )
                else:
                    eng = nc.sync if si % 2 == 0 else nc.scalar
                    eng.dma_start(out=at[:], in_=a_in[g * 128 : (g + 1) * 128, :])
                ats[si] = at

            for si, (ch, p) in enumerate(sched):
                if p == 0:
                    aggs[ch] = agg_pool.tile([128, IC], F32, name=f"agg{ch}", tag="agg")
                agg = aggs[ch]
                if si not in ats:
                    issue_group(si)
                at = ats.pop(si)
                extra_dma(si, ch, p, issue_group)
                if double_row:
                    for r in range(NCORES):
                        for u in range(NT // 2):
                            nc.tensor.matmul(
                                agg[:],
                                lhsT=stath[r][p][:, 2 * u : 2 * u + 2, :],
                                rhs=at[:, r * NT + 2 * u : r * NT + 2 * u + 2, :],
                                start=(p == 0 and r == 0 and u == 0),
                                stop=(
                                    p == NPIECE - 1
                                    and r == NCORES - 1
                                    and u == NT // 2 - 1
                                ),
                                perf_mode=DR,
                            )
                else:
                    for r in range(NCORES):
                        for t in range(NT):
                            jr = p * NT + t
                            nc.tensor.matmul(
                                agg[:],
                                lhsT=statx[r][:, jr * 128 : (jr + 1) * 128],
                                rhs=at[:, r * NT + t : r * NT + t + 1, :],
                                start=(p == 0 and r == 0 and t == 0),
                                stop=(
                                    p == NPIECE - 1
                                    and r == NCORES - 1
                                    and t == NT - 1
                                ),
                            )
                if p == NPIECE - 1:
                    mt = m_pool.tile([128, IC], F32R, name="mt", tag="mt")
                    nc.vector.tensor_copy(out=mt[:], in_=agg[:])
                    for it in range(IC // 128):
                        lp = lin_pool.tile([128, D], F32, name="lp", tag="lp")
                        nc.tensor.matmul(
                            lp[:],
                            lhsT=mt[:, it * 128 : (it + 1) * 128],
                            rhs=w_sb[:],
                            start=True,
                            stop=True,
                        )
                        write_out(ch, it, lp)
                    chunk_done(ch)

        # ---- layer 0 ----
        def write_l0(ch, it, lp):
            ht = h_pool.tile([128, D], E4, name="ht0", tag="ht0")
            nc.scalar.activation(ht[:], lp[:], relu)
            # scalar hwdge queue: the gpsimd queue carries the CC doorbell
            # writes, which can block on a backed-up CC stream (slow cross-
            # core barrier) and would stall ht recycling -> the PE stream
            nc.scalar.dma_start(out=h_tb[ch][:, it * 128 : (it + 1) * 128], in_=ht[:])

        def gather(ch):
            nc.gpsimd.collective_compute(
                "AllGather",
                mybir.AluOpType.bypass,
                replica_groups=[list(range(NCORES))],
                ins=[h_tb[ch][:]],
                outs=[h_ag[ch][:]],
            )

        def extra_l0(si, ch, p, issue_group):
            # interleave the remaining X loads with the split first groups:
            # queue order sync = sx0,sx2,g0h,sx4,g1h,sx6,g2..., mirrored on
            # scalar — supplies track the PE's rank-by-rank demand in block 0
            if si == 0:
                load_statx(4)
                load_statx(5)
                issue_group(1)
                load_statx(6)
                load_statx(7)

        layer(w0_sb, write_l0, gather, piece_outer=False, double_row=False,
              extra_dma=extra_l0)

        # ---- layer 1 ----
        def write_l1(ch, it, lp):
            ht = h_pool.tile([128, D], F32, name="ht1", tag="ht1")
            nc.scalar.activation(ht[:], lp[:], relu)
            nc.scalar.dma_start(
                out=h_out[ch * IC + it * 128 : ch * IC + (it + 1) * 128, :], in_=ht[:]
            )

        # stationary-H piece loads, gpsimd only, emitted AFTER every gather
        # trigger: a piece-p load waits on gather-p completing, and putting
        # any such wait between h writes and a later gather trigger (or on
        # the A-stream queues) stalls that path on straggler cores
        for p in range(NPIECE):
            for r in range(NCORES):
                nc.gpsimd.dma_start(
                    out=stath[r][p][:], in_=h_ag[p][r * 128 : (r + 1) * 128, :]
                )

        layer(w1_sb, write_l1, lambda ch: None, piece_outer=True, double_row=True,
              extra_dma=lambda si, ch, p, issue_group: None)

    nc.finalize()
    return nc


def _tile_stat(X):
    """[16384, 128] -> [1024, 2048] stationary layout."""
    return np.ascontiguousarray(
        X.reshape(NCORES, 16, 128, D).transpose(0, 2, 1, 3).reshape(NCORES * 128, ROWS)
    )


def shard_inputs(A_norm, X, W0, W1):
    """Host-side shard prep. Returns per-core input maps."""
    import ml_dtypes

    bf16 = ml_dtypes.bfloat16
    e4 = ml_dtypes.float8_e4m3

    x_t = _tile_stat(np.asarray(X, np.float32)).astype(bf16)
    # fold the fp8 pre-scales into the weights:
    #   psum_l0 = S_A*(A@X);      w0' = W0*S_H/S_A  -> h_tb = S_H*H0 (e4m3)
    #   psum_l1 = S_A*S_H*(A@H0); w1' = W1/(S_A*S_H)
    w0 = np.ascontiguousarray(np.asarray(W0, np.float32) * np.float32(S_H / S_A))
    w1 = np.ascontiguousarray(np.asarray(W1, np.float32) / np.float32(S_A * S_H))

    in_maps = []
    for c in range(NCORES):
        a_tc = np.asarray(A_norm[c * ROWS : (c + 1) * ROWS, :], np.float32).T
        a8 = np.clip(a_tc * np.float32(S_A), 0.0, 240.0).astype(e4)
        # [16384, 2048] -> chunk-major groups (see a_in comment)
        a_pre = np.ascontiguousarray(
            a8.reshape(NCORES, NPIECE, NT, 128, NCH, IC)
            .transpose(4, 1, 3, 0, 2, 5)
            .reshape(NG * 128, NB * IC)
        )
        in_maps.append({"a0": a_pre, "x0": x_t, "w0": w0, "w1": w1})
    return in_maps


_CACHED = {}


def kernel(A_norm, X, W0, W1):
    A_norm = np.ascontiguousarray(A_norm, dtype=np.float32)
    X = np.ascontiguousarray(X, dtype=np.float32)
    W0 = np.ascontiguousarray(W0, dtype=np.float32)
    W1 = np.ascontiguousarray(W1, dtype=np.float32)

    from concourse.bass_utils import run_bass_kernel_spmd

    if PRECISION not in _CACHED:
        _CACHED[PRECISION] = build_gcn()
    nc = _CACHED[PRECISION]

    in_maps = shard_inputs(A_norm, X, W0, W1)
    res = run_bass_kernel_spmd(nc, in_maps, core_ids=list(range(NCORES)))
    return np.concatenate([res.results[c]["h_out"] for c in range(NCORES)], axis=0)
